# revision 8
# baseline (speedup 1.0000x reference)
"""IterNorm (iterative whitening normalization) Bass kernel for 8 TRN2 cores.

Reference (hardcoded shapes): X (64, 256, 56, 56) f32; g=4 groups of d=64
channels; m = 64*56*56 = 200704; Sigma = eps*I + (1/m) xc xc^T per group;
5 Newton-Schulz iters -> whitening wm; out = (wm @ xc) * weight + bias.

Sharding: data-parallel over batch B (8 b's per core). Per core:
  phase 1: stream local shard (natural layout [channels, hw]), accumulate
           per-group S = x x^T (PE transpose chunks + accumulating matmuls)
           and channel sums (DVE reduce). The first NRES of 16 (b, half)
           tiles stay resident in SBUF for phase 2.
  all-reduce 132KB of packed stats; replicated stats math + Newton-Schulz
           (2 groups packed per 128-tile via tile_position diag blocks).
  phase 2: apply out = W' @ x + offset (W' folds weight*sqrt(rTr)*P, offset
           folds bias - W' @ mean); resident tiles first, then reload rest.
"""

import numpy as np

B, C, H, W = 64, 256, 56, 56
HW = H * W               # 3136
G, D = 4, 64             # groups, channels/group
NCORES = 8
BS = B // NCORES         # 8 batches per core
M = B * HW               # 200704 (full reduction length)
EPS = 1e-5
T_ITERS = 5

NCH = 128                # transpose chunk width (hw)
FULL_CHUNKS = HW // NCH  # 24
TAIL = HW - FULL_CHUNKS * NCH  # 64
APPLY_N = 392            # apply matmul free dim; 8 * 392 = 3136
OUT_CHUNK = 1568         # output store chunk (2 per tile)
NRES = 10                # resident (b, half) tiles kept in SBUF

_CACHE = {}


def _build_nc(single_core_sim=False, repeat=1, bf16_stats=False):
    import concourse.bacc as bacc
    import concourse.tile as tile
    from concourse import mybir

    f32 = mybir.dt.float32
    bf16 = mybir.dt.bfloat16
    st_dt = bf16 if bf16_stats else f32
    AX = mybir.AxisListType.X
    ADD = mybir.AluOpType.add
    SUB = mybir.AluOpType.subtract
    MULT = mybir.AluOpType.mult

    nc = bacc.Bacc(
        "TRN2",
        target_bir_lowering=False,
        debug=False,
        enable_asserts=False,
        num_devices=1 if single_core_sim else NCORES,
    )
    Xd = nc.dram_tensor("X", [BS, C, HW], f32, kind="ExternalInput").ap()
    Wd = nc.dram_tensor("weight", [C], f32, kind="ExternalInput").ap()
    Bd = nc.dram_tensor("bias", [C], f32, kind="ExternalInput").ap()
    Od = nc.dram_tensor("out", [BS, C, HW], f32, kind="ExternalOutput").ap()

    pairs = [(b, h) for b in range(BS) for h in range(2)]
    blksl = [slice(0, 64), slice(64, 128)]
    widths = [NCH] * FULL_CHUNKS + [TAIL]
    offs = [i * NCH for i in range(FULL_CHUNKS + 1)]
    blocks = [list(range(4 * kb, min(4 * kb + 4, 25))) for kb in range(7)]
    inv_m = 1.0 / float(M)

    with tile.TileContext(nc) as tc:
        with (
            tc.tile_pool(name="consts", bufs=1) as consts,
            tc.tile_pool(name="res", bufs=NRES) as res,
            tc.tile_pool(name="p1x", bufs=2) as p1x,
            tc.tile_pool(name="p1t", bufs=4) as p1t,
            tc.tile_pool(name="statsp", bufs=2) as statsp,
            tc.tile_pool(name="nss", bufs=3) as nss,
            tc.tile_pool(name="apo", bufs=3) as apo,
            tc.tile_pool(name="dram", bufs=1, space="DRAM") as dram,
        ):
            # ---- constants (inline data, DMA'd once) ----
            id_np = np.eye(128, dtype=np.float32)
            gm_np = np.zeros((128, 2), dtype=np.float32)
            gm_np[0:64, 0] = 1.0
            gm_np[64:128, 1] = 1.0
            identity_d = nc.inline_tensor(id_np, name="identity_c")
            epsI_d = nc.inline_tensor(EPS * id_np, name="epsI_c")
            gmask_d = nc.inline_tensor(gm_np, name="gmask_c")
            ones_d = nc.inline_tensor(np.ones((1, 128), dtype=np.float32),
                                      name="ones_c")
            identity = consts.tile([128, 128], f32)
            nc.sync.dma_start(out=identity, in_=identity_d.ap())
            epsI = consts.tile([128, 128], f32)
            nc.sync.dma_start(out=epsI, in_=epsI_d.ap())
            gmask = consts.tile([128, 2], f32)
            nc.sync.dma_start(out=gmask, in_=gmask_d.ap())
            ones_row = consts.tile([1, 128], f32)
            nc.sync.dma_start(out=ones_row, in_=ones_d.ap())
            wrow = consts.tile([1, C], f32)
            nc.sync.dma_start(out=wrow, in_=Wd[None, :])
            bcol = consts.tile([128, 2], f32)
            nc.sync.dma_start(out=bcol[:, 0:1], in_=Bd[0:128][:, None])
            nc.sync.dma_start(out=bcol[:, 1:2], in_=Bd[128:256][:, None])

            for _rep in range(repeat):
                # ---- phase 1: local channel sums + covariance ----
                sums = statsp.tile([128, 2 * BS], f32, name="sums")
                cc_in = statsp.tile([128, 258], f32, name="cc_in")
                x_tiles = {}

                with tc.tile_pool(name="p1tp", bufs=4, space="PSUM") as p1tp, \
                     tc.tile_pool(name="covp", bufs=2, space="PSUM") as covp:
                    cov = [covp.tile([128, 128], f32, tag=f"cov{h}",
                                     name=f"cov{h}") for h in range(2)]
                    copy_eng = 0
                    for i, (b, h) in enumerate(pairs):
                        hs = slice(h * 128, (h + 1) * 128)
                        if i < NRES:
                            xt = res.tile([128, HW], f32, tag="rxt", name="rxt")
                            x_tiles[(b, h)] = xt
                        else:
                            xt = p1x.tile([128, HW], f32, tag="xt", name="xt")
                        nc.sync.dma_start(out=xt, in_=Xd[b, hs, :])
                        nc.vector.reduce_sum(
                            out=sums[:, i:i + 1], in_=xt, axis=AX)
                        for kb, blk_chunks in enumerate(blocks):
                            pt = p1tp.tile([128, 512], f32, tag="pt", name="pt")
                            st = p1t.tile([128, 512], st_dt, tag="st", name="st")
                            for j, cidx in enumerate(blk_chunks):
                                kw = widths[cidx]
                                nc.tensor.transpose(
                                    pt[0:kw, j * NCH:j * NCH + 128],
                                    xt[:, offs[cidx]:offs[cidx] + kw],
                                    identity,
                                )
                            eng = nc.vector if copy_eng % 2 == 0 else nc.scalar
                            copy_eng += 1
                            kwall = 128 if len(blk_chunks) == 4 else widths[blk_chunks[0]]
                            fwall = len(blk_chunks) * NCH
                            if eng is nc.vector:
                                eng.tensor_copy(st[0:kwall, 0:fwall],
                                                pt[0:kwall, 0:fwall])
                            else:
                                eng.copy(st[0:kwall, 0:fwall],
                                         pt[0:kwall, 0:fwall])
                            for j, cidx in enumerate(blk_chunks):
                                kw = widths[cidx]
                                first = (i == 0) and (cidx == 0)
                                last = (i == len(pairs) - 1) and (cidx == 24)
                                nc.tensor.matmul(
                                    cov[h],
                                    st[0:kw, j * NCH:j * NCH + 128],
                                    st[0:kw, j * NCH:j * NCH + 128],
                                    start=first, stop=last,
                                )
                    nc.vector.tensor_copy(cc_in[:, 0:128], cov[0])
                    nc.vector.tensor_copy(cc_in[:, 128:256], cov[1])
                    nc.vector.reduce_sum(out=cc_in[:, 256:257],
                                         in_=sums[:, 0:16:2], axis=AX)
                    nc.vector.reduce_sum(out=cc_in[:, 257:258],
                                         in_=sums[:, 1:16:2], axis=AX)

                # ---- all-reduce ----
                bounce_in = dram.tile([128, 258], f32, tag="bin", name="bin")
                bounce_out = dram.tile([128, 258], f32, tag="bout", name="bout")
                nc.sync.dma_start(out=bounce_in, in_=cc_in)
                if single_core_sim:
                    nc.sync.dma_start(out=bounce_out, in_=bounce_in)
                else:
                    nc.gpsimd.collective_compute(
                        "AllReduce",
                        mybir.AluOpType.add,
                        replica_groups=[list(range(NCORES))],
                        ins=[bounce_in.opt()],
                        outs=[bounce_out.opt()],
                    )
                stats = statsp.tile([128, 258], f32, name="stats")
                nc.sync.dma_start(out=stats, in_=bounce_out)

                # ---- stats math + Newton-Schulz ----
                traces = statsp.tile([1, 4], f32, name="traces")
                mean_col = [statsp.tile([128, 1], f32, tag=f"mc{h}",
                                        name=f"mc{h}") for h in range(2)]
                Sig = [nss.tile([128, 128], f32, tag=f"sig{h}",
                                name=f"sig{h}") for h in range(2)]

                with tc.tile_pool(name="nsp", bufs=4, space="PSUM") as nsp:
                    for h in range(2):
                        nc.vector.tensor_scalar(
                            out=mean_col[h], in0=stats[:, 256 + h:257 + h],
                            scalar1=inv_m, scalar2=None, op0=MULT)
                        pmr = nsp.tile([128, 128], f32, tag="nsmisc", bufs=2,
                                       name="pmr")
                        nc.tensor.transpose(pmr[0:1, 0:128], mean_col[h],
                                            identity)
                        mrow = statsp.tile([1, 128], f32, tag=f"mr{h}",
                                           name=f"mr{h}")
                        nc.vector.tensor_copy(mrow, pmr[0:1, 0:128])
                        pouter = nsp.tile([128, 128], f32, tag="nsmisc", bufs=2,
                                          name="pouter")
                        nc.tensor.matmul(pouter, mrow, mrow, start=True,
                                         stop=True)
                        nc.vector.tensor_scalar(
                            out=Sig[h], in0=stats[:, h * 128:(h + 1) * 128],
                            scalar1=inv_m, scalar2=None, op0=MULT)
                        nc.vector.tensor_tensor(
                            out=Sig[h], in0=Sig[h], in1=pouter, op=SUB)
                        nc.vector.tensor_tensor(
                            out=Sig[h], in0=Sig[h], in1=epsI, op=ADD)
                        dtmp = nss.tile([128, 128], f32, tag="dtmp", name="dtmp")
                        nc.vector.tensor_tensor(out=dtmp, in0=Sig[h],
                                                in1=identity, op=MULT)
                        dcol = statsp.tile([128, 1], f32, tag=f"dc{h}",
                                           name=f"dc{h}")
                        nc.vector.reduce_sum(out=dcol, in_=dtmp, axis=AX)
                        ptr = nsp.tile([128, 128], f32, tag="nsmisc", bufs=2,
                                       name="ptr")
                        nc.tensor.matmul(ptr[0:1, 0:2], dcol, gmask,
                                         start=True, stop=True)
                        nc.vector.tensor_copy(traces[0:1, 2 * h:2 * h + 2],
                                              ptr[0:1, 0:2])

                    rtr = statsp.tile([1, 4], f32, name="rtr")
                    nc.vector.reciprocal(rtr, traces)
                    srtr = statsp.tile([1, 4], f32, name="srtr")
                    nc.scalar.sqrt(srtr, rtr)
                    pbc = nsp.tile([128, 128], f32, tag="nsmisc", bufs=2,
                                   name="pbc")
                    nc.tensor.matmul(pbc[:, 0:4], ones_row, rtr,
                                     start=True, stop=True)
                    nc.tensor.matmul(pbc[:, 4:8], ones_row, srtr,
                                     start=True, stop=True)
                    bc = statsp.tile([128, 8], f32, name="bc")
                    nc.vector.tensor_copy(bc, pbc[:, 0:8])
                    pwb = nsp.tile([128, 256], f32, tag="nsmisc", bufs=2,
                                   name="pwb")
                    nc.tensor.matmul(pwb, ones_row, wrow, start=True, stop=True)
                    wbc = nss.tile([128, 256], f32, tag="wbc", name="wbc")
                    nc.vector.tensor_copy(wbc, pwb)

                    rtr_col = [statsp.tile([128, 1], f32, tag=f"rc{h}",
                                           name=f"rc{h}") for h in range(2)]
                    srtr_col = [statsp.tile([128, 1], f32, tag=f"sc{h}",
                                            name=f"sc{h}") for h in range(2)]
                    wm = [nss.tile([128, 128], f32, tag=f"wm{h}",
                                   name=f"wm{h}") for h in range(2)]
                    offs_col = [statsp.tile([128, 1], f32, tag=f"of{h}",
                                            name=f"of{h}") for h in range(2)]

                    for h in range(2):
                        sel = statsp.tile([128, 2], f32, tag=f"sel{h}",
                                          name=f"sel{h}")
                        nc.vector.tensor_tensor(
                            out=sel, in0=bc[:, 2 * h:2 * h + 2], in1=gmask,
                            op=MULT)
                        nc.vector.reduce_sum(out=rtr_col[h], in_=sel, axis=AX)
                        sel2 = statsp.tile([128, 2], f32, tag=f"sel2{h}",
                                           name=f"sel2{h}")
                        nc.vector.tensor_tensor(
                            out=sel2, in0=bc[:, 4 + 2 * h:6 + 2 * h],
                            in1=gmask, op=MULT)
                        nc.vector.reduce_sum(out=srtr_col[h], in_=sel2, axis=AX)

                        sigN = nss.tile([128, 128], f32, tag=f"sn{h}",
                                        name=f"sn{h}")
                        nc.vector.tensor_scalar(
                            out=sigN, in0=Sig[h], scalar1=rtr_col[h],
                            scalar2=None, op0=MULT)

                        P = nss.tile([128, 128], f32, tag=f"P{h}", name=f"P{h}")
                        nc.vector.tensor_copy(P, identity)
                        for t in range(T_ITERS):
                            psA = nsp.tile([128, 128], f32, tag="nsmm", bufs=3,
                                           name="psA")
                            Asb = nss.tile([128, 128], f32, tag="Asb",
                                           name="Asb")
                            for k, sl in enumerate(blksl):
                                nc.tensor.matmul(
                                    psA[sl, sl], P[sl, sl], P[sl, sl],
                                    start=True, stop=True,
                                    tile_position=(64 * k, 64 * k))
                            for sl in blksl:
                                nc.vector.tensor_copy(Asb[sl, sl], psA[sl, sl])
                            psB = nsp.tile([128, 128], f32, tag="nsmm", bufs=3,
                                           name="psB")
                            Bsb = nss.tile([128, 128], f32, tag="Bsb",
                                           name="Bsb")
                            for k, sl in enumerate(blksl):
                                nc.tensor.matmul(
                                    psB[sl, sl], Asb[sl, sl], P[sl, sl],
                                    start=True, stop=True,
                                    tile_position=(64 * k, 64 * k))
                            for sl in blksl:
                                nc.vector.tensor_copy(Bsb[sl, sl], psB[sl, sl])
                            psC = nsp.tile([128, 128], f32, tag="nsmm", bufs=3,
                                           name="psC")
                            Csb = nss.tile([128, 128], f32, tag="Csb",
                                           name="Csb")
                            for k, sl in enumerate(blksl):
                                nc.tensor.matmul(
                                    psC[sl, sl], Bsb[sl, sl], sigN[sl, sl],
                                    start=True, stop=True,
                                    tile_position=(64 * k, 64 * k))
                            for sl in blksl:
                                nc.scalar.mul(Csb[sl, sl], psC[sl, sl], -0.5)
                            Pn = nss.tile([128, 128], f32, tag=f"P{h}",
                                          name=f"Pn{h}")
                            for sl in blksl:
                                nc.vector.tensor_scalar(
                                    out=Pn[sl, sl], in0=P[sl, sl],
                                    scalar1=1.5, scalar2=None, op0=MULT)
                                nc.vector.tensor_tensor(
                                    out=Pn[sl, sl], in0=Pn[sl, sl],
                                    in1=Csb[sl, sl], op=ADD)
                            P = Pn
                        for sl in blksl:
                            nc.vector.tensor_scalar(
                                out=wm[h][sl, sl], in0=P[sl, sl],
                                scalar1=srtr_col[h][sl, :], scalar2=None,
                                op0=MULT)
                            nc.vector.tensor_tensor(
                                out=wm[h][sl, sl], in0=wm[h][sl, sl],
                                in1=wbc[sl, h * 128 + sl.start:h * 128 + sl.stop],
                                op=MULT)
                        poff = nsp.tile([128, 128], f32, tag="nsmisc", bufs=2,
                                        name="poff")
                        for k, sl in enumerate(blksl):
                            nc.tensor.matmul(
                                poff[sl, 0:1], wm[h][sl, sl],
                                mean_col[h][sl, :],
                                start=True, stop=True,
                                tile_position=(64 * k, 64 * k))
                        nc.vector.tensor_tensor(
                            out=offs_col[h], in0=bcol[:, h:h + 1],
                            in1=poff[:, 0:1], op=SUB)

                # ---- phase 2: apply (resident tiles first, then reload) ----
                with tc.tile_pool(name="app", bufs=4, space="PSUM") as app:
                    for i, (b, h) in enumerate(pairs):
                        hs = slice(h * 128, (h + 1) * 128)
                        if i < NRES:
                            xt = x_tiles[(b, h)]
                        else:
                            xt = p1x.tile([128, HW], f32, tag="xt", name="xt2")
                            nc.sync.dma_start(out=xt, in_=Xd[b, hs, :])
                        for oc in range(HW // OUT_CHUNK):
                            aot = apo.tile([128, OUT_CHUNK], f32, tag="aot",
                                           name="aot")
                            for k in range(OUT_CHUNK // APPLY_N):
                                gk = oc * (OUT_CHUNK // APPLY_N) + k
                                nsl = slice(gk * APPLY_N, (gk + 1) * APPLY_N)
                                osl = slice(k * APPLY_N, (k + 1) * APPLY_N)
                                pap = app.tile([128, APPLY_N], f32, tag="pap",
                                               name="pap")
                                for kk, sl in enumerate(blksl):
                                    nc.tensor.matmul(
                                        pap[sl, :], wm[h][sl, sl], xt[sl, nsl],
                                        start=True, stop=True,
                                        tile_position=(64 * kk, 64 * kk))
                                nc.vector.tensor_scalar(
                                    out=aot[:, osl], in0=pap,
                                    scalar1=offs_col[h], scalar2=None, op0=ADD)
                            nc.sync.dma_start(
                                out=Od[b, hs,
                                       oc * OUT_CHUNK:(oc + 1) * OUT_CHUNK],
                                in_=aot)
    nc.compile()
    return nc


def kernel(X, weight, bias):
    from concourse.bass_utils import run_bass_kernel_spmd

    if "nc" not in _CACHE:
        _CACHE["nc"] = _build_nc()
    nc = _CACHE["nc"]

    X = np.ascontiguousarray(np.asarray(X, dtype=np.float32)).reshape(B, C, HW)
    w = np.ascontiguousarray(np.asarray(weight, dtype=np.float32)).reshape(C)
    bb = np.ascontiguousarray(np.asarray(bias, dtype=np.float32)).reshape(C)
    in_maps = [
        {"X": np.ascontiguousarray(X[i * BS:(i + 1) * BS]),
         "weight": w, "bias": bb}
        for i in range(NCORES)
    ]
    res = run_bass_kernel_spmd(nc, in_maps, core_ids=list(range(NCORES)))
    _CACHE["last_result"] = res
    out = np.concatenate([r["out"] for r in res.results], axis=0)
    return out.reshape(B, C, H, W)


# revision 9
# speedup vs baseline: 3.5122x; 3.5122x over previous
"""IterNorm (iterative whitening normalization) Bass kernel for 8 TRN2 cores.

Reference (hardcoded shapes): X (64, 256, 56, 56) f32; g=4 groups of d=64
channels; m = 64*56*56 = 200704; Sigma = eps*I + (1/m) xc xc^T per group;
5 Newton-Schulz iters -> whitening wm; out = (wm @ xc) * weight + bias.

Sharding: data-parallel over batch B (8 b's per core). Per core:
  phase 1: stream local shard (natural layout [channels, hw]), accumulate
           per-group S = x x^T (PE transpose chunks + accumulating matmuls)
           and channel sums (DVE reduce). The first NRES of 16 (b, half)
           tiles stay resident in SBUF for phase 2.
  all-reduce 132KB of packed stats; replicated stats math + Newton-Schulz
           (2 groups packed per 128-tile via tile_position diag blocks).
  phase 2: apply out = W' @ x + offset (W' folds weight*sqrt(rTr)*P, offset
           folds bias - W' @ mean); resident tiles first, then reload rest.
"""

import numpy as np

B, C, H, W = 64, 256, 56, 56
HW = H * W               # 3136
G, D = 4, 64             # groups, channels/group
NCORES = 8
BS = B // NCORES         # 8 batches per core
M = B * HW               # 200704 (full reduction length)
EPS = 1e-5
T_ITERS = 5

NCH = 128                # transpose chunk width (hw)
FULL_CHUNKS = HW // NCH  # 24
TAIL = HW - FULL_CHUNKS * NCH  # 64
APPLY_N = 392            # apply matmul free dim; 8 * 392 = 3136
OUT_CHUNK = 1568         # output store chunk (2 per tile)
NRES = 10                # resident (b, half) tiles kept in SBUF

_CACHE = {}


def _build_nc(single_core_sim=False, repeat=1, bf16_stats=False, nres=NRES):
    import concourse.bacc as bacc
    import concourse.tile as tile
    from concourse import mybir

    f32 = mybir.dt.float32
    bf16 = mybir.dt.bfloat16
    st_dt = bf16 if bf16_stats else f32
    AX = mybir.AxisListType.X
    ADD = mybir.AluOpType.add
    SUB = mybir.AluOpType.subtract
    MULT = mybir.AluOpType.mult

    nc = bacc.Bacc(
        "TRN2",
        target_bir_lowering=False,
        debug=False,
        enable_asserts=False,
        num_devices=1 if single_core_sim else NCORES,
    )
    Xd = nc.dram_tensor("X", [BS, C, HW], f32, kind="ExternalInput").ap()
    Wd = nc.dram_tensor("weight", [C], f32, kind="ExternalInput").ap()
    Bd = nc.dram_tensor("bias", [C], f32, kind="ExternalInput").ap()
    Od = nc.dram_tensor("out", [BS, C, HW], f32, kind="ExternalOutput").ap()

    pairs = [(b, h) for b in range(BS) for h in range(2)]
    blksl = [slice(0, 64), slice(64, 128)]
    widths = [NCH] * FULL_CHUNKS + [TAIL]
    offs = [i * NCH for i in range(FULL_CHUNKS + 1)]
    blocks = [list(range(4 * kb, min(4 * kb + 4, 25))) for kb in range(7)]
    inv_m = 1.0 / float(M)

    with tile.TileContext(nc) as tc:
        with (
            tc.tile_pool(name="consts", bufs=1) as consts,
            tc.tile_pool(name="res", bufs=max(nres, 1)) as res,
            tc.tile_pool(name="p1x", bufs=2) as p1x,
            tc.tile_pool(name="p1t", bufs=4) as p1t,
            tc.tile_pool(name="statsp", bufs=2) as statsp,
            tc.tile_pool(name="nss", bufs=3) as nss,
            tc.tile_pool(name="apo", bufs=3) as apo,
            tc.tile_pool(name="dram", bufs=1, space="DRAM") as dram,
        ):
            # ---- constants (inline data, DMA'd once) ----
            id_np = np.eye(128, dtype=np.float32)
            gm_np = np.zeros((128, 2), dtype=np.float32)
            gm_np[0:64, 0] = 1.0
            gm_np[64:128, 1] = 1.0
            identity_d = nc.inline_tensor(id_np, name="identity_c")
            epsI_d = nc.inline_tensor(EPS * id_np, name="epsI_c")
            gmask_d = nc.inline_tensor(gm_np, name="gmask_c")
            ones_d = nc.inline_tensor(np.ones((1, 128), dtype=np.float32),
                                      name="ones_c")
            identity = consts.tile([128, 128], f32)
            nc.sync.dma_start(out=identity, in_=identity_d.ap())
            epsI = consts.tile([128, 128], f32)
            nc.sync.dma_start(out=epsI, in_=epsI_d.ap())
            gmask = consts.tile([128, 2], f32)
            nc.sync.dma_start(out=gmask, in_=gmask_d.ap())
            ones_row = consts.tile([1, 128], f32)
            nc.sync.dma_start(out=ones_row, in_=ones_d.ap())
            wrow = consts.tile([1, C], f32)
            nc.sync.dma_start(out=wrow, in_=Wd[None, :])
            bcol = consts.tile([128, 2], f32)
            nc.sync.dma_start(out=bcol[:, 0:1], in_=Bd[0:128][:, None])
            nc.sync.dma_start(out=bcol[:, 1:2], in_=Bd[128:256][:, None])

            for _rep in range(repeat):
                # ---- phase 1: local channel sums + covariance ----
                sums = statsp.tile([128, 2 * BS], f32, name="sums")
                cc_in = statsp.tile([128, 258], f32, name="cc_in")
                x_tiles = {}

                with tc.tile_pool(name="p1tp", bufs=4, space="PSUM") as p1tp, \
                     tc.tile_pool(name="covp", bufs=2, space="PSUM") as covp:
                    cov = [covp.tile([128, 128], f32, tag=f"cov{h}",
                                     name=f"cov{h}") for h in range(2)]
                    copy_eng = 0
                    for i, (b, h) in enumerate(pairs):
                        hs = slice(h * 128, (h + 1) * 128)
                        if i < nres:
                            xt = res.tile([128, HW], f32, tag="rxt", name="rxt")
                            x_tiles[(b, h)] = xt
                        else:
                            xt = p1x.tile([128, HW], f32, tag="xt", name="xt")
                        nc.sync.dma_start(out=xt, in_=Xd[b, hs, :])
                        nc.vector.reduce_sum(
                            out=sums[:, i:i + 1], in_=xt, axis=AX)
                        for kb, blk_chunks in enumerate(blocks):
                            pt = p1tp.tile([128, 512], f32, tag="pt", name="pt")
                            st = p1t.tile([128, 512], st_dt, tag="st", name="st")
                            for j, cidx in enumerate(blk_chunks):
                                kw = widths[cidx]
                                nc.tensor.transpose(
                                    pt[0:kw, j * NCH:j * NCH + 128],
                                    xt[:, offs[cidx]:offs[cidx] + kw],
                                    identity,
                                )
                            eng = nc.vector if copy_eng % 2 == 0 else nc.scalar
                            copy_eng += 1
                            kwall = 128 if len(blk_chunks) == 4 else widths[blk_chunks[0]]
                            fwall = len(blk_chunks) * NCH
                            if eng is nc.vector:
                                eng.tensor_copy(st[0:kwall, 0:fwall],
                                                pt[0:kwall, 0:fwall])
                            else:
                                eng.copy(st[0:kwall, 0:fwall],
                                         pt[0:kwall, 0:fwall])
                            for j, cidx in enumerate(blk_chunks):
                                kw = widths[cidx]
                                first = (i == 0) and (cidx == 0)
                                last = (i == len(pairs) - 1) and (cidx == 24)
                                nc.tensor.matmul(
                                    cov[h],
                                    st[0:kw, j * NCH:j * NCH + 128],
                                    st[0:kw, j * NCH:j * NCH + 128],
                                    start=first, stop=last,
                                )
                    nc.vector.tensor_copy(cc_in[:, 0:128], cov[0])
                    nc.vector.tensor_copy(cc_in[:, 128:256], cov[1])
                    nc.vector.reduce_sum(out=cc_in[:, 256:257],
                                         in_=sums[:, 0:16:2], axis=AX)
                    nc.vector.reduce_sum(out=cc_in[:, 257:258],
                                         in_=sums[:, 1:16:2], axis=AX)

                # ---- all-reduce ----
                bounce_in = dram.tile([128, 258], f32, tag="bin", name="bin")
                bounce_out = dram.tile([128, 258], f32, tag="bout", name="bout")
                nc.sync.dma_start(out=bounce_in, in_=cc_in)
                if single_core_sim:
                    nc.sync.dma_start(out=bounce_out, in_=bounce_in)
                else:
                    nc.gpsimd.collective_compute(
                        "AllReduce",
                        mybir.AluOpType.add,
                        replica_groups=[list(range(NCORES))],
                        ins=[bounce_in.opt()],
                        outs=[bounce_out.opt()],
                    )
                stats = statsp.tile([128, 258], f32, name="stats")
                nc.sync.dma_start(out=stats, in_=bounce_out)

                # ---- stats math + Newton-Schulz ----
                traces = statsp.tile([1, 4], f32, name="traces")
                mean_col = [statsp.tile([128, 1], f32, tag=f"mc{h}",
                                        name=f"mc{h}") for h in range(2)]
                Sig = [nss.tile([128, 128], f32, tag=f"sig{h}",
                                name=f"sig{h}") for h in range(2)]

                with tc.tile_pool(name="nsp", bufs=4, space="PSUM") as nsp:
                    for h in range(2):
                        nc.vector.tensor_scalar(
                            out=mean_col[h], in0=stats[:, 256 + h:257 + h],
                            scalar1=inv_m, scalar2=None, op0=MULT)
                        pmr = nsp.tile([128, 128], f32, tag="nsmisc", bufs=2,
                                       name="pmr")
                        nc.tensor.transpose(pmr[0:1, 0:128], mean_col[h],
                                            identity)
                        mrow = statsp.tile([1, 128], f32, tag=f"mr{h}",
                                           name=f"mr{h}")
                        nc.vector.tensor_copy(mrow, pmr[0:1, 0:128])
                        pouter = nsp.tile([128, 128], f32, tag="nsmisc", bufs=2,
                                          name="pouter")
                        nc.tensor.matmul(pouter, mrow, mrow, start=True,
                                         stop=True)
                        nc.vector.tensor_scalar(
                            out=Sig[h], in0=stats[:, h * 128:(h + 1) * 128],
                            scalar1=inv_m, scalar2=None, op0=MULT)
                        nc.vector.tensor_tensor(
                            out=Sig[h], in0=Sig[h], in1=pouter, op=SUB)
                        nc.vector.tensor_tensor(
                            out=Sig[h], in0=Sig[h], in1=epsI, op=ADD)
                        dtmp = nss.tile([128, 128], f32, tag="dtmp", name="dtmp")
                        nc.vector.tensor_tensor(out=dtmp, in0=Sig[h],
                                                in1=identity, op=MULT)
                        dcol = statsp.tile([128, 1], f32, tag=f"dc{h}",
                                           name=f"dc{h}")
                        nc.vector.reduce_sum(out=dcol, in_=dtmp, axis=AX)
                        ptr = nsp.tile([128, 128], f32, tag="nsmisc", bufs=2,
                                       name="ptr")
                        nc.tensor.matmul(ptr[0:1, 0:2], dcol, gmask,
                                         start=True, stop=True)
                        nc.vector.tensor_copy(traces[0:1, 2 * h:2 * h + 2],
                                              ptr[0:1, 0:2])

                    rtr = statsp.tile([1, 4], f32, name="rtr")
                    nc.vector.reciprocal(rtr, traces)
                    srtr = statsp.tile([1, 4], f32, name="srtr")
                    nc.scalar.sqrt(srtr, rtr)
                    pbc = nsp.tile([128, 128], f32, tag="nsmisc", bufs=2,
                                   name="pbc")
                    nc.tensor.matmul(pbc[:, 0:4], ones_row, rtr,
                                     start=True, stop=True)
                    nc.tensor.matmul(pbc[:, 4:8], ones_row, srtr,
                                     start=True, stop=True)
                    bc = statsp.tile([128, 8], f32, name="bc")
                    nc.vector.tensor_copy(bc, pbc[:, 0:8])
                    pwb = nsp.tile([128, 256], f32, tag="nsmisc", bufs=2,
                                   name="pwb")
                    nc.tensor.matmul(pwb, ones_row, wrow, start=True, stop=True)
                    wbc = nss.tile([128, 256], f32, tag="wbc", name="wbc")
                    nc.vector.tensor_copy(wbc, pwb)

                    rtr_col = [statsp.tile([128, 1], f32, tag=f"rc{h}",
                                           name=f"rc{h}") for h in range(2)]
                    srtr_col = [statsp.tile([128, 1], f32, tag=f"sc{h}",
                                            name=f"sc{h}") for h in range(2)]
                    wm = [nss.tile([128, 128], f32, tag=f"wm{h}",
                                   name=f"wm{h}") for h in range(2)]
                    offs_col = [statsp.tile([128, 1], f32, tag=f"of{h}",
                                            name=f"of{h}") for h in range(2)]

                    for h in range(2):
                        sel = statsp.tile([128, 2], f32, tag=f"sel{h}",
                                          name=f"sel{h}")
                        nc.vector.tensor_tensor(
                            out=sel, in0=bc[:, 2 * h:2 * h + 2], in1=gmask,
                            op=MULT)
                        nc.vector.reduce_sum(out=rtr_col[h], in_=sel, axis=AX)
                        sel2 = statsp.tile([128, 2], f32, tag=f"sel2{h}",
                                           name=f"sel2{h}")
                        nc.vector.tensor_tensor(
                            out=sel2, in0=bc[:, 4 + 2 * h:6 + 2 * h],
                            in1=gmask, op=MULT)
                        nc.vector.reduce_sum(out=srtr_col[h], in_=sel2, axis=AX)

                        sigN = nss.tile([128, 128], f32, tag=f"sn{h}",
                                        name=f"sn{h}")
                        nc.vector.tensor_scalar(
                            out=sigN, in0=Sig[h], scalar1=rtr_col[h],
                            scalar2=None, op0=MULT)

                        P = nss.tile([128, 128], f32, tag=f"P{h}", name=f"P{h}")
                        nc.vector.tensor_copy(P, identity)
                        for t in range(T_ITERS):
                            psA = nsp.tile([128, 128], f32, tag="nsmm", bufs=3,
                                           name="psA")
                            Asb = nss.tile([128, 128], f32, tag="Asb",
                                           name="Asb")
                            for k, sl in enumerate(blksl):
                                nc.tensor.matmul(
                                    psA[sl, sl], P[sl, sl], P[sl, sl],
                                    start=True, stop=True,
                                    tile_position=(64 * k, 64 * k))
                            for sl in blksl:
                                nc.vector.tensor_copy(Asb[sl, sl], psA[sl, sl])
                            psB = nsp.tile([128, 128], f32, tag="nsmm", bufs=3,
                                           name="psB")
                            Bsb = nss.tile([128, 128], f32, tag="Bsb",
                                           name="Bsb")
                            for k, sl in enumerate(blksl):
                                nc.tensor.matmul(
                                    psB[sl, sl], Asb[sl, sl], P[sl, sl],
                                    start=True, stop=True,
                                    tile_position=(64 * k, 64 * k))
                            for sl in blksl:
                                nc.vector.tensor_copy(Bsb[sl, sl], psB[sl, sl])
                            psC = nsp.tile([128, 128], f32, tag="nsmm", bufs=3,
                                           name="psC")
                            Csb = nss.tile([128, 128], f32, tag="Csb",
                                           name="Csb")
                            for k, sl in enumerate(blksl):
                                nc.tensor.matmul(
                                    psC[sl, sl], Bsb[sl, sl], sigN[sl, sl],
                                    start=True, stop=True,
                                    tile_position=(64 * k, 64 * k))
                            for sl in blksl:
                                nc.scalar.mul(Csb[sl, sl], psC[sl, sl], -0.5)
                            Pn = nss.tile([128, 128], f32, tag=f"P{h}",
                                          name=f"Pn{h}")
                            for sl in blksl:
                                nc.vector.tensor_scalar(
                                    out=Pn[sl, sl], in0=P[sl, sl],
                                    scalar1=1.5, scalar2=None, op0=MULT)
                                nc.vector.tensor_tensor(
                                    out=Pn[sl, sl], in0=Pn[sl, sl],
                                    in1=Csb[sl, sl], op=ADD)
                            P = Pn
                        for sl in blksl:
                            nc.vector.tensor_scalar(
                                out=wm[h][sl, sl], in0=P[sl, sl],
                                scalar1=srtr_col[h][sl, :], scalar2=None,
                                op0=MULT)
                            nc.vector.tensor_tensor(
                                out=wm[h][sl, sl], in0=wm[h][sl, sl],
                                in1=wbc[sl, h * 128 + sl.start:h * 128 + sl.stop],
                                op=MULT)
                        poff = nsp.tile([128, 128], f32, tag="nsmisc", bufs=2,
                                        name="poff")
                        for k, sl in enumerate(blksl):
                            nc.tensor.matmul(
                                poff[sl, 0:1], wm[h][sl, sl],
                                mean_col[h][sl, :],
                                start=True, stop=True,
                                tile_position=(64 * k, 64 * k))
                        nc.vector.tensor_tensor(
                            out=offs_col[h], in0=bcol[:, h:h + 1],
                            in1=poff[:, 0:1], op=SUB)

                # ---- phase 2: apply (resident tiles first, then reload) ----
                with tc.tile_pool(name="app", bufs=4, space="PSUM") as app:
                    for i, (b, h) in enumerate(pairs):
                        hs = slice(h * 128, (h + 1) * 128)
                        if i < nres:
                            xt = x_tiles[(b, h)]
                        else:
                            xt = p1x.tile([128, HW], f32, tag="xt", name="xt2")
                            nc.sync.dma_start(out=xt, in_=Xd[b, hs, :])
                        for oc in range(HW // OUT_CHUNK):
                            aot = apo.tile([128, OUT_CHUNK], f32, tag="aot",
                                           name="aot")
                            for k in range(OUT_CHUNK // APPLY_N):
                                gk = oc * (OUT_CHUNK // APPLY_N) + k
                                nsl = slice(gk * APPLY_N, (gk + 1) * APPLY_N)
                                osl = slice(k * APPLY_N, (k + 1) * APPLY_N)
                                pap = app.tile([128, APPLY_N], f32, tag="pap",
                                               name="pap")
                                for kk, sl in enumerate(blksl):
                                    nc.tensor.matmul(
                                        pap[sl, :], wm[h][sl, sl], xt[sl, nsl],
                                        start=True, stop=True,
                                        tile_position=(64 * kk, 64 * kk))
                                nc.vector.tensor_scalar(
                                    out=aot[:, osl], in0=pap,
                                    scalar1=offs_col[h], scalar2=None, op0=ADD)
                            nc.sync.dma_start(
                                out=Od[b, hs,
                                       oc * OUT_CHUNK:(oc + 1) * OUT_CHUNK],
                                in_=aot)
                if repeat > 1 and _rep < repeat - 1:
                    tc.strict_bb_all_engine_barrier()
    nc.compile()
    return nc


def kernel(X, weight, bias):
    from concourse.bass_utils import run_bass_kernel_spmd

    if "nc" not in _CACHE:
        _CACHE["nc"] = _build_nc()
    nc = _CACHE["nc"]

    X = np.ascontiguousarray(np.asarray(X, dtype=np.float32)).reshape(B, C, HW)
    w = np.ascontiguousarray(np.asarray(weight, dtype=np.float32)).reshape(C)
    bb = np.ascontiguousarray(np.asarray(bias, dtype=np.float32)).reshape(C)
    in_maps = [
        {"X": np.ascontiguousarray(X[i * BS:(i + 1) * BS]),
         "weight": w, "bias": bb}
        for i in range(NCORES)
    ]
    res = run_bass_kernel_spmd(nc, in_maps, core_ids=list(range(NCORES)))
    _CACHE["last_result"] = res
    out = np.concatenate([r["out"] for r in res.results], axis=0)
    return out.reshape(B, C, H, W)
